# revision 5
# baseline (speedup 1.0000x reference)
"""CRX gate (controlled-RX on 12-qubit state batch) as a Trainium2 Bass kernel.

Problem: y = U @ x with U the CRX(angle) unitary, DIM=2, NQ=12, control
qubit 0 (stride 2048), target qubit 1 (stride 1024), D=4096, B=128.

Semantics (derived from the reference):
  - rows d in [0, 2048): control bit 0 -> identity (y = x)
  - rows d in [2048, 3072) pair with d+1024; with c=cos(angle/2),
    s=sin(angle/2):
      y[d]      = c*x[d]      - 1j*s*x[d+1024]
      y[d+1024] = -1j*s*x[d]  + c*x[d+1024]

Strategy: batch (column) sharding across 8 NeuronCores, 16 columns each.
Only the rotated half (rows 2048:4096) is shipped to the device; the
identity half is a host passthrough. Per core the device sees one
[128, 516] f32 tile: cols 0:3 carry (c, s, -s) replicated per partition
(so the NEFF is angle-independent and is compiled exactly once per
process), col 3 pad, cols 4:516 the rotated amplitudes. Host-side column
packing puts each rotation pair (value and its partner) in contiguous
chunks, so the device program is a 2-chunk pipeline:
  in-DMA chunk (SP/HWDGE) -> t = c*X; o = +-s*swap(X) + t (DVE, fused
  scalar_tensor_tensor) -> out-DMA chunk (ACT/HWDGE)

Raw Bass (no TileContext): the Tile tail drain accumulates >1 sem wait,
which this container's walrus codegen rejects ("Too many sync wait
commands"), so synchronization is manual.
"""

import numpy as np

_NCORES = 8
_D = 4096
_B = 128
_BC = _B // _NCORES  # 16 batch columns per core
_HALF = 2048
_Q = 1024
_W = 512             # data columns per core
_S = 4               # leading scalar/pad columns: c, s, -s, pad
_N_CHUNKS = 2

LAST_RESULTS = None   # BassKernelResults of the most recent run (for test.py)
_NC_CACHE = None      # angle-independent Bass module, built once per process


def _perm() -> np.ndarray:
    """Column permutation: packed[:, j] = quarters[:, perm[j]].

    Quarter layout (col base): Ar=0, Bi=128, Br=256, Ai=384. Each chunk is
    [L | R] with L, R matching H-col slices of a (left, right) pair, pairs
    being (Ar, Bi) and (Br, Ai).
    """
    w = _W // _N_CHUNKS
    h = w // 2
    cols = []
    for lbase, rbase in ((0, 128), (256, 384)):
        for j in range(128 // h):
            cols.extend(range(lbase + j * h, lbase + (j + 1) * h))
            cols.extend(range(rbase + j * h, rbase + (j + 1) * h))
    return np.asarray(cols)


def _build_bass():
    import concourse.bass as bass
    import concourse.mybir as mybir

    w = _W // _N_CHUNKS  # data cols per chunk
    h = w // 2

    nc = bass.Bass("TRN2")
    x = nc.dram_tensor("x", [128, _S + _W], mybir.dt.float32, kind="ExternalInput")
    y = nc.dram_tensor("y", [128, _W], mybir.dt.float32, kind="ExternalOutput")

    with (
        nc.sbuf_tensor([128, _S + _W], mybir.dt.float32) as xt,
        nc.sbuf_tensor([128, _W], mybir.dt.float32) as t,
        nc.sbuf_tensor([128, _W], mybir.dt.float32) as o,
        nc.semaphore() as dsem_in,
        nc.semaphore() as vsem,
        nc.semaphore() as dsem_out,
        nc.Block() as block,
    ):
        cs = xt[:, 0:1]   # c   per partition
        ss = xt[:, 1:2]   # s
        ns = xt[:, 2:3]   # -s

        @block.sync
        def _(sync):
            # chunk 0 carries the scalar columns too
            sync.dma_start(xt[:, 0 : _S + w], x[:, 0 : _S + w]).then_inc(dsem_in, 16)
            for i in range(1, _N_CHUNKS):
                sync.dma_start(
                    xt[:, _S + i * w : _S + (i + 1) * w],
                    x[:, _S + i * w : _S + (i + 1) * w],
                ).then_inc(dsem_in, 16)

        @block.vector
        def _(vector):
            for i in range(_N_CHUNKS):
                lo, mid, hi = i * w, i * w + h, (i + 1) * w
                vector.wait_ge(dsem_in, 16 * (i + 1))
                nc.vector.tensor_scalar_mul(
                    t[:, lo:hi], xt[:, _S + lo : _S + hi], cs
                )
                nc.vector.scalar_tensor_tensor(
                    out=o[:, lo:mid],
                    in0=xt[:, _S + mid : _S + hi],
                    scalar=ss,
                    in1=t[:, lo:mid],
                    op0=mybir.AluOpType.mult,
                    op1=mybir.AluOpType.add,
                )
                nc.vector.scalar_tensor_tensor(
                    out=o[:, mid:hi],
                    in0=xt[:, _S + lo : _S + mid],
                    scalar=ns,
                    in1=t[:, mid:hi],
                    op0=mybir.AluOpType.mult,
                    op1=mybir.AluOpType.add,
                ).then_inc(vsem, 1)

        @block.scalar
        def _(scalar):
            for i in range(_N_CHUNKS):
                scalar.wait_ge(vsem, i + 1)
                nc.scalar.dma_start(
                    y[:, i * w : (i + 1) * w], o[:, i * w : (i + 1) * w]
                ).then_inc(dsem_out, 16)
            scalar.wait_ge(dsem_out, 16 * _N_CHUNKS)

    return nc


def _get_nc():
    global _NC_CACHE
    if _NC_CACHE is None:
        _NC_CACHE = _build_bass()
    return _NC_CACHE


def _prep_in_maps(x: np.ndarray, c: float, s: float):
    A = x[_HALF : _HALF + _Q]  # (1024, 128)
    Bv = x[_HALF + _Q :]       # (1024, 128)
    perm = _perm()
    in_maps = []
    for k in range(_NCORES):
        sl = slice(k * _BC, (k + 1) * _BC)
        M = np.stack(
            [A[:, sl].real, Bv[:, sl].imag, Bv[:, sl].real, A[:, sl].imag]
        )  # (4, 1024, BC) f32
        # quarters layout: row d' = n*128 + p -> [p, quarter*128 + n*16 + b]
        Xq = M.reshape(4, 8, 128, _BC).transpose(2, 0, 1, 3).reshape(128, _W)
        Xk = np.empty((128, _S + _W), dtype=np.float32)
        Xk[:, 0] = c
        Xk[:, 1] = s
        Xk[:, 2] = -s
        Xk[:, 3] = 0.0
        Xk[:, _S:] = Xq[:, perm]
        in_maps.append({"x": Xk})
    return in_maps


def _unpack_out(y: np.ndarray, results):
    perm = _perm()
    for k in range(_NCORES):
        sl = slice(k * _BC, (k + 1) * _BC)
        Yp = results[k]["y"]
        Yq = np.empty_like(Yp)
        Yq[:, perm] = Yp
        Yk = Yq.reshape(128, 4, 8, _BC).transpose(1, 2, 0, 3).reshape(4, _Q, _BC)
        y[_HALF : _HALF + _Q, sl] = Yk[0] + 1j * Yk[3]
        y[_HALF + _Q :, sl] = Yk[2] + 1j * Yk[1]


def kernel(x, angle):
    global LAST_RESULTS
    from concourse.bass_utils import run_bass_kernel_spmd

    x = np.asarray(x)
    angle = np.asarray(angle)
    assert x.shape == (_D, _B) and x.dtype == np.complex64, (x.shape, x.dtype)

    theta = 0.5 * float(np.float32(angle.reshape(-1)[0]))
    c = float(np.cos(theta))
    s = float(np.sin(theta))

    y = np.empty((_D, _B), dtype=np.complex64)
    y[:_HALF] = x[:_HALF]  # control bit 0: identity

    in_maps = _prep_in_maps(x, c, s)
    nc = _get_nc()
    res = run_bass_kernel_spmd(nc, in_maps, core_ids=list(range(_NCORES)))
    LAST_RESULTS = res
    _unpack_out(y, res.results)
    return y


# revision 7
# speedup vs baseline: 1.0251x; 1.0251x over previous
"""CRX gate (controlled-RX on 12-qubit state batch) as a Trainium2 Bass kernel.

Problem: y = U @ x with U the CRX(angle) unitary, DIM=2, NQ=12, control
qubit 0 (stride 2048), target qubit 1 (stride 1024), D=4096, B=128.

Semantics (derived from the reference):
  - rows d in [0, 2048): control bit 0 -> identity (y = x)
  - rows d in [2048, 3072) pair with d+1024; with c=cos(angle/2),
    s=sin(angle/2):
      y[d]      = c*x[d]      - 1j*s*x[d+1024]
      y[d+1024] = -1j*s*x[d]  + c*x[d+1024]

Strategy: batch (column) sharding across 8 NeuronCores, 16 columns each.
Only the rotated half (rows 2048:4096) is shipped to the device; the
identity half is a host passthrough. Per core the device sees one
[128, 516] f32 tile: cols 0:3 carry (c, s, -s) replicated per partition
(so the NEFF is angle-independent and is compiled exactly once per
process), col 3 pad, cols 4:516 the rotated amplitudes. Host-side column
packing puts each rotation pair (value and its partner) in contiguous
chunks, so the device program is a 2-chunk pipeline:
  in-DMA chunk (SP/HWDGE) -> t = c*X; o = +-s*swap(X) + t (DVE, fused
  scalar_tensor_tensor) -> out-DMA chunk (ACT/HWDGE)

Raw Bass (no TileContext): the Tile tail drain accumulates >1 sem wait,
which this container's walrus codegen rejects ("Too many sync wait
commands"), so synchronization is manual.
"""

import numpy as np

_NCORES = 8
_D = 4096
_B = 128
_BC = _B // _NCORES  # 16 batch columns per core
_HALF = 2048
_Q = 1024
_W = 512             # data columns per core
_S = 4               # leading scalar/pad columns: c, s, -s, pad
_N_CHUNKS = 2

LAST_RESULTS = None   # BassKernelResults of the most recent run (for test.py)
_NC_CACHE = None      # angle-independent Bass module, built once per process


def _perm() -> np.ndarray:
    """Column permutation: packed[:, j] = quarters[:, perm[j]].

    Quarter layout (col base): Ar=0, Bi=128, Br=256, Ai=384. Each chunk is
    [L | R] with L, R matching H-col slices of a (left, right) pair, pairs
    being (Ar, Bi) and (Br, Ai).
    """
    w = _W // _N_CHUNKS
    h = w // 2
    cols = []
    for lbase, rbase in ((0, 128), (256, 384)):
        for j in range(128 // h):
            cols.extend(range(lbase + j * h, lbase + (j + 1) * h))
            cols.extend(range(rbase + j * h, rbase + (j + 1) * h))
    return np.asarray(cols)


def _build_bass():
    import concourse.bass as bass
    import concourse.mybir as mybir

    w = _W // _N_CHUNKS  # data cols per chunk
    h = w // 2

    nc = bass.Bass("TRN2")
    x = nc.dram_tensor("x", [128, _S + _W], mybir.dt.float32, kind="ExternalInput")
    y = nc.dram_tensor("y", [128, _W], mybir.dt.float32, kind="ExternalOutput")

    with (
        nc.sbuf_tensor([128, _S + _W], mybir.dt.float32) as xt,
        nc.sbuf_tensor([128, _W], mybir.dt.float32) as t,
        nc.sbuf_tensor([128, _W], mybir.dt.float32) as o,
        nc.semaphore() as dsem_in,
        nc.semaphore() as vsem,
        nc.semaphore() as dsem_out,
        nc.Block() as block,
    ):
        cs = xt[:, 0:1]   # c   per partition
        ss = xt[:, 1:2]   # s
        ns = xt[:, 2:3]   # -s

        @block.sync
        def _(sync):
            # chunk 0 carries the scalar columns too
            sync.dma_start(xt[:, 0 : _S + w], x[:, 0 : _S + w]).then_inc(dsem_in, 16)
            for i in range(1, _N_CHUNKS):
                sync.dma_start(
                    xt[:, _S + i * w : _S + (i + 1) * w],
                    x[:, _S + i * w : _S + (i + 1) * w],
                ).then_inc(dsem_in, 16)
            # out-DMAs also on SP: cheaper dge path than ACT in practice and
            # the issue sequence (in0, in1, wait, out0, wait, out1) never
            # blocks an in-DMA behind a compute wait.
            for i in range(_N_CHUNKS):
                sync.wait_ge(vsem, i + 1)
                sync.dma_start(
                    y[:, i * w : (i + 1) * w], o[:, i * w : (i + 1) * w]
                ).then_inc(dsem_out, 16)
            sync.wait_ge(dsem_out, 16 * _N_CHUNKS)

        @block.vector
        def _(vector):
            for i in range(_N_CHUNKS):
                lo, mid, hi = i * w, i * w + h, (i + 1) * w
                vector.wait_ge(dsem_in, 16 * (i + 1))
                nc.vector.tensor_scalar_mul(
                    t[:, lo:hi], xt[:, _S + lo : _S + hi], cs
                )
                nc.vector.scalar_tensor_tensor(
                    out=o[:, lo:mid],
                    in0=xt[:, _S + mid : _S + hi],
                    scalar=ss,
                    in1=t[:, lo:mid],
                    op0=mybir.AluOpType.mult,
                    op1=mybir.AluOpType.add,
                )
                nc.vector.scalar_tensor_tensor(
                    out=o[:, mid:hi],
                    in0=xt[:, _S + lo : _S + mid],
                    scalar=ns,
                    in1=t[:, mid:hi],
                    op0=mybir.AluOpType.mult,
                    op1=mybir.AluOpType.add,
                ).then_inc(vsem, 1)

    return nc


def _get_nc():
    global _NC_CACHE
    if _NC_CACHE is None:
        _NC_CACHE = _build_bass()
    return _NC_CACHE


def _prep_in_maps(x: np.ndarray, c: float, s: float):
    A = x[_HALF : _HALF + _Q]  # (1024, 128)
    Bv = x[_HALF + _Q :]       # (1024, 128)
    perm = _perm()
    in_maps = []
    for k in range(_NCORES):
        sl = slice(k * _BC, (k + 1) * _BC)
        M = np.stack(
            [A[:, sl].real, Bv[:, sl].imag, Bv[:, sl].real, A[:, sl].imag]
        )  # (4, 1024, BC) f32
        # quarters layout: row d' = n*128 + p -> [p, quarter*128 + n*16 + b]
        Xq = M.reshape(4, 8, 128, _BC).transpose(2, 0, 1, 3).reshape(128, _W)
        Xk = np.empty((128, _S + _W), dtype=np.float32)
        Xk[:, 0] = c
        Xk[:, 1] = s
        Xk[:, 2] = -s
        Xk[:, 3] = 0.0
        Xk[:, _S:] = Xq[:, perm]
        in_maps.append({"x": Xk})
    return in_maps


def _unpack_out(y: np.ndarray, results):
    perm = _perm()
    for k in range(_NCORES):
        sl = slice(k * _BC, (k + 1) * _BC)
        Yp = results[k]["y"]
        Yq = np.empty_like(Yp)
        Yq[:, perm] = Yp
        Yk = Yq.reshape(128, 4, 8, _BC).transpose(1, 2, 0, 3).reshape(4, _Q, _BC)
        y[_HALF : _HALF + _Q, sl] = Yk[0] + 1j * Yk[3]
        y[_HALF + _Q :, sl] = Yk[2] + 1j * Yk[1]


def kernel(x, angle):
    global LAST_RESULTS
    from concourse.bass_utils import run_bass_kernel_spmd

    x = np.asarray(x)
    angle = np.asarray(angle)
    assert x.shape == (_D, _B) and x.dtype == np.complex64, (x.shape, x.dtype)

    theta = 0.5 * float(np.float32(angle.reshape(-1)[0]))
    c = float(np.cos(theta))
    s = float(np.sin(theta))

    y = np.empty((_D, _B), dtype=np.complex64)
    y[:_HALF] = x[:_HALF]  # control bit 0: identity

    in_maps = _prep_in_maps(x, c, s)
    nc = _get_nc()
    res = run_bass_kernel_spmd(nc, in_maps, core_ids=list(range(_NCORES)))
    LAST_RESULTS = res
    _unpack_out(y, res.results)
    return y
